# revision 36
# baseline (speedup 1.0000x reference)
"""Trainium2 Bass kernel for nn_AdditiveAttention (B=4, Q=K=2048, D=1024).

Math: scores[b,q,k] = (sum_d q[b,q,d] + sum_d v[b,k,d]) / sqrt(D) + mask bias.
The q-row term is constant along k, so it cancels in the softmax exactly:
    weights[b,q,k] = mask[b,q,k] * e[b,k] / denom[b,q]
    e[b,k]     = exp(sum_d value[b,k,d] / sqrt(D))
    denom[b,q] = sum_k mask[b,q,k] * e[b,k]
    context    = (mask @ (e * value)) / denom        (per batch)
(`query` only affects the reference output through float rounding.)

Sharding: 8 cores = (batch b in 0..3) x (query half h in 0..1); each core
computes a (1024, :) slice of both outputs for its batch. No collectives.

Compute dtype: bf16 matmul (the 0/1 mask operand is exactly representable),
fp32 accumulate in PSUM. Masks ship as int8 and are cast to bf16 in flight
by gpsimd (SWDGE) DMAs on their own ring; value ships bf16 on the sync ring;
outputs ship bf16 and are upcast to f32 on the host (~0.5% rel err vs the
2e-2 tolerance).

Orchestration notes (all engine streams are in-order; the layout below keeps
every cross-engine dependency off the critical path):
  - denominators come from N=1 PE matmuls accumulated alongside the context
    tiles (8 ns each), so PSUM evacuation never waits on the e-broadcast;
  - the e-vector broadcast bounces through DRAM in "kappa" order
    (kappa = p*KT + t) so both DMAs are dense; the host permutes the
    weights-path mask input and inverse-permutes the weights output;
  - the first three query tiles' accumulation chains are interleaved so the
    PE tracks the input DMA stream densely during the ramp (8 PSUM banks:
    3x c0 + 3x c1 + 2x dn).
Measured: ~87 us NEFF exec (neuron-profile), rel err ~3e-3.
"""

import os
import sys

import numpy as np
import ml_dtypes

for _p in ("/opt/trn_rl_repo", "/root/.axon_site/_ro/trn_rl_repo"):
    if os.path.isdir(_p) and _p not in sys.path:
        sys.path.append(_p)

import concourse.bacc as bacc
import concourse.mybir as mybir
from concourse.tile import TileContext
from concourse.bass_utils import run_bass_kernel_spmd

B, Q, K, D = 4, 2048, 2048, 1024
P = 128
NCORES = 8
QSH = Q // 2          # query rows per core
KT = K // P           # 16 k tiles
QT = QSH // P         # 8 q tiles per core
F32 = mybir.dt.float32
BF16 = mybir.dt.bfloat16
I8 = mybir.dt.int8

_cache = {}


def _build():
    nc = bacc.Bacc()
    value = nc.declare_dram_parameter("value", [K, D], BF16, isOutput=False)
    maskT = nc.declare_dram_parameter("maskT", [K, QSH], I8, isOutput=False)
    maskT0 = nc.declare_dram_parameter("maskT0", [P, QSH], BF16, isOutput=False)
    maskN = nc.declare_dram_parameter("maskN", [QSH, K], I8, isOutput=False)
    ctx_o = nc.declare_dram_parameter("ctx", [QSH, D], BF16, isOutput=True)
    wts_o = nc.declare_dram_parameter("wts", [QSH, K], BF16, isOutput=True)
    e_dram = nc.dram_tensor("e_scratch", [1, K], BF16)

    AF = mybir.ActivationFunctionType
    ALU = mybir.AluOpType

    with TileContext(nc) as tc:
        with (
            tc.tile_pool(name="vpool", bufs=1) as vpool,
            tc.tile_pool(name="sval", bufs=1) as spool,
            tc.tile_pool(name="stats", bufs=1) as stats,
            tc.tile_pool(name="scr", bufs=2) as scr,
            tc.tile_pool(name="masks", bufs=1) as masks,
            tc.tile_pool(name="mepool", bufs=3) as mep,
            tc.tile_pool(name="wsb", bufs=3) as wsb,
            tc.tile_pool(name="csb", bufs=3) as csb,
            tc.tile_pool(name="small", bufs=8) as small,
            tc.tile_pool(name="psum", bufs=3, space="PSUM") as pp,
            tc.tile_pool(name="psumd", bufs=2, space="PSUM") as ppd,
        ):
            # Preload the ACT exp table while the first DMAs are in flight.
            warm = stats.tile([P, 1], F32, tag="warm")
            nc.vector.memset(warm[:], 0.0)
            nc.scalar.activation(warm[:], warm[:], AF.Exp)

            ks = stats.tile([P, KT], F32, tag="ks")
            e_f = stats.tile([P, KT], F32, tag="ef")
            e_b = stats.tile([P, KT], BF16, tag="eb")
            ebc = stats.tile([P, K], BF16, tag="ebc")

            mTa = masks.tile([P, KT, QSH], BF16, tag="mT")
            mT3 = maskT.rearrange("(t p) q -> p t q", p=P)
            mNa = masks.tile([P, QT, K], BF16, tag="mN")
            mN3 = maskN.rearrange("(j p) k -> p j k", p=P)

            # Stage A: per-tile {maskT, value} loads; ksum split ACT/DVE;
            # e = exp(ksum/32) in bf16; sval = e * value (bf16).
            v3 = value.rearrange("(t p) d -> t p d", p=P)
            # Tile 0 of maskT rides first on the fast sync ring (bf16, no
            # cast) so the very first matmul is not gated by the gpsimd
            # ring's slow first dispatch.
            nc.sync.dma_start(out=mTa[:, 0, :], in_=maskT0[:, :])
            vts, svs = [], []
            for t in range(KT):
                vt = vpool.tile([P, D], BF16, tag=f"v{t}")
                nc.sync.dma_start(out=vt[:], in_=v3[t])
                vts.append(vt)
                if t > 0:
                    nc.gpsimd.dma_start(out=mTa[:, t, :], in_=mT3[:, t, :])
            for t in range(KT):
                if t % 2 == 0 and t > 0:
                    dummy = scr.tile([P, D], BF16, tag="scratch")
                    nc.scalar.activation(
                        dummy[:], vts[t][:], AF.Copy, accum_out=ks[:, t : t + 1]
                    )
                else:
                    nc.vector.tensor_reduce(
                        ks[:, t : t + 1], vts[t][:], axis=mybir.AxisListType.X,
                        op=ALU.add,
                    )
                nc.scalar.activation(
                    e_b[:, t : t + 1], ks[:, t : t + 1], AF.Exp, scale=1.0 / 32.0
                )
                nc.scalar.activation(
                    e_f[:, t : t + 1], ks[:, t : t + 1], AF.Exp, scale=1.0 / 32.0
                )
                sv = spool.tile([P, D], BF16, tag=f"s{t}")
                nc.vector.tensor_scalar_mul(sv[:], vts[t][:], e_f[:, t : t + 1])
                svs.append(sv)

            # maskN after the critical value+maskT stream (weights path only)
            for j in range(QT):
                nc.gpsimd.dma_start(out=mNa[:, j, :], in_=mN3[:, j, :])

            # ebc[q_part, kappa] = e broadcast along partitions, via a dense
            # DRAM bounce in kappa order (kappa = p*KT + t; host permutes the
            # weights path to match). gpsimd ring: off both input and output
            # HWDGE rings, so its e_b wait blocks no other traffic.
            nc.gpsimd.dma_start(
                out=e_dram[0, :].rearrange("(p t) -> p t", p=P), in_=e_b[:]
            )
            nc.gpsimd.dma_start(
                out=ebc[:], in_=e_dram[0:1, :].partition_broadcast(P)
            )

            # Stage B: context matmuls + PE-side denominators (N=1 matmuls,
            # ready exactly when each context PSUM completes, independent of
            # the late e-broadcast), 3-way interleaved ramp, weights path
            # (DVE multiply + ACT scale) threaded behind the evacuations.
            cts, dns, rs = {}, {}, {}

            def alloc_ct(j):
                cts[j] = (
                    pp.tile([P, 512], F32, tag="c0", name=f"c0_{j}"),
                    pp.tile([P, 512], F32, tag="c1", name=f"c1_{j}"),
                )

            def alloc_dn(j):
                dns[j] = ppd.tile([P, 1], F32, tag="dn", name=f"dn_{j}")

            def mm_ct(j, t):
                qs = slice(j * P, (j + 1) * P)
                st, sp = (t == 0), (t == KT - 1)
                c0, c1 = cts[j]
                lhsT = mTa[:, t, qs]
                nc.tensor.matmul(c0[:], lhsT, svs[t][:, 0:512], start=st, stop=sp)
                nc.tensor.matmul(c1[:], lhsT, svs[t][:, 512:1024], start=st, stop=sp)

            def mm_dn(j, t):
                qs = slice(j * P, (j + 1) * P)
                st, sp = (t == 0), (t == KT - 1)
                nc.tensor.matmul(
                    dns[j][:], mTa[:, t, qs], e_b[:, t : t + 1], start=st, stop=sp
                )

            def evac(j):
                qs = slice(j * P, (j + 1) * P)
                c0, c1 = cts.pop(j)
                dn = dns.pop(j)
                r = small.tile([P, 1], F32, tag="r", name=f"r_{j}")
                nc.vector.reciprocal(r[:], dn[:])
                cs = csb.tile([P, D], BF16, tag="cs", name=f"cs_{j}")
                nc.vector.tensor_scalar_mul(cs[:, 0:512], c0[:], r[:])
                nc.vector.tensor_scalar_mul(cs[:, 512:1024], c1[:], r[:])
                nc.scalar.dma_start(out=ctx_o[qs, :], in_=cs[:])
                rs[j] = r

            def wts_path(j, me=None):
                qs = slice(j * P, (j + 1) * P)
                if me is None:
                    me = mep.tile([P, K], BF16, tag="me", name=f"me_{j}")
                    nc.vector.tensor_tensor(
                        out=me[:], in0=mNa[:, j, :], in1=ebc[:], op=ALU.mult
                    )
                w = wsb.tile([P, K], BF16, tag="w", name=f"w_{j}")
                if j == QT - 1:
                    nc.vector.tensor_scalar_mul(w[:], me[:], rs[j][:])
                else:
                    nc.scalar.activation(w[:], me[:], AF.Copy, scale=rs[j][:])
                nc.scalar.dma_start(out=wts_o[qs, :], in_=w[:])

            RAMP = (0, 1, 2)
            for j in RAMP:
                alloc_ct(j)
            alloc_dn(0)
            alloc_dn(1)
            for t in range(KT):
                for j in RAMP:
                    mm_ct(j, t)
                    if j < 2:
                        mm_dn(j, t)
            alloc_dn(2)
            for t in range(KT):
                mm_dn(2, t)
            for j in RAMP:
                evac(j)
            for j in RAMP:
                wts_path(j)
            for j in range(3, QT):
                alloc_ct(j)
                alloc_dn(j)
                for t in range(KT):
                    mm_ct(j, t)
                    mm_dn(j, t)
                me = mep.tile([P, K], BF16, tag="me", name=f"me_{j}")
                nc.vector.tensor_tensor(
                    out=me[:], in0=mNa[:, j, :], in1=ebc[:], op=ALU.mult
                )
                evac(j)
                wts_path(j, me=me)

    nc.compile()
    return nc


def kernel(query, value, attention_mask):
    nc = _cache.get("nc")
    if nc is None:
        nc = _cache["nc"] = _build()

    value = np.asarray(value, dtype=np.float32)
    mask = np.asarray(attention_mask)

    # kappa-order: kappa = p*KT + t  <->  k = t*P + p. The weights path runs
    # in kappa order on-chip (dense e-vector bounce); host permutes maskN in
    # and un-permutes the wts output.
    kap = np.arange(K)
    k_of_kappa = (kap % KT) * P + kap // KT
    inv_kappa = (kap % P) * KT + kap // P  # kappa index of natural column k

    in_maps = []
    for c in range(NCORES):
        b, h = divmod(c, 2)
        msub = mask[b, h * QSH : (h + 1) * QSH, :]
        in_maps.append(
            {
                "value": value[b].astype(ml_dtypes.bfloat16),
                "maskT": np.ascontiguousarray(msub.T).astype(np.int8),
                "maskT0": np.ascontiguousarray(msub.T[0:P]).astype(
                    ml_dtypes.bfloat16
                ),
                "maskN": np.ascontiguousarray(
                    msub[:, k_of_kappa]
                ).astype(np.int8),
            }
        )

    res = run_bass_kernel_spmd(nc, in_maps, core_ids=list(range(NCORES)))
    _cache["last_results"] = res

    ctx = np.empty((B, Q, D), np.float32)
    wts = np.empty((B, Q, K), np.float32)
    for c in range(NCORES):
        b, h = divmod(c, 2)
        ctx[b, h * QSH : (h + 1) * QSH] = res.results[c]["ctx"].astype(np.float32)
        wts[b, h * QSH : (h + 1) * QSH] = (
            res.results[c]["wts"].astype(np.float32)[:, inv_kappa]
        )
    return ctx, wts



# revision 37
# speedup vs baseline: 1.0283x; 1.0283x over previous
"""Trainium2 Bass kernel for nn_AdditiveAttention (B=4, Q=K=2048, D=1024).

Math: scores[b,q,k] = (sum_d q[b,q,d] + sum_d v[b,k,d]) / sqrt(D) + mask bias.
The q-row term is constant along k, so it cancels in the softmax exactly:
    weights[b,q,k] = mask[b,q,k] * e[b,k] / denom[b,q]
    e[b,k]     = exp(sum_d value[b,k,d] / sqrt(D))
    denom[b,q] = sum_k mask[b,q,k] * e[b,k]
    context    = (mask @ (e * value)) / denom        (per batch)
(`query` only affects the reference output through float rounding.)

Sharding: 8 cores = (batch b in 0..3) x (query half h in 0..1); each core
computes a (1024, :) slice of both outputs for its batch. No collectives.

Compute dtype: bf16 matmul (the 0/1 mask operand is exactly representable),
fp32 accumulate in PSUM. Masks ship as int8 and are cast to bf16 in flight
by gpsimd (SWDGE) DMAs on their own ring; value ships bf16 on the sync ring;
outputs ship bf16 and are upcast to f32 on the host (~0.5% rel err vs the
2e-2 tolerance).

Orchestration notes (all engine streams are in-order; the layout below keeps
every cross-engine dependency off the critical path):
  - denominators come from N=1 PE matmuls accumulated alongside the context
    tiles (8 ns each), so PSUM evacuation never waits on the e-broadcast;
  - the e-vector broadcast bounces through DRAM in "kappa" order
    (kappa = p*KT + t) so both DMAs are dense; the host permutes the
    weights-path mask input and inverse-permutes the weights output;
  - the first three query tiles' accumulation chains are interleaved so the
    PE tracks the input DMA stream densely during the ramp (8 PSUM banks:
    3x c0 + 3x c1 + 2x dn).
Measured: ~87 us NEFF exec (neuron-profile), rel err ~3e-3.
"""

import os
import sys

import numpy as np
import ml_dtypes

for _p in ("/opt/trn_rl_repo", "/root/.axon_site/_ro/trn_rl_repo"):
    if os.path.isdir(_p) and _p not in sys.path:
        sys.path.append(_p)

import concourse.bacc as bacc
import concourse.mybir as mybir
from concourse.tile import TileContext
from concourse.bass_utils import run_bass_kernel_spmd

B, Q, K, D = 4, 2048, 2048, 1024
P = 128
NCORES = 8
QSH = Q // 2          # query rows per core
KT = K // P           # 16 k tiles
QT = QSH // P         # 8 q tiles per core
F32 = mybir.dt.float32
BF16 = mybir.dt.bfloat16
I8 = mybir.dt.int8

_cache = {}


def _build():
    nc = bacc.Bacc()
    value = nc.declare_dram_parameter("value", [K, D], BF16, isOutput=False)
    maskT = nc.declare_dram_parameter("maskT", [K, QSH], I8, isOutput=False)
    maskN = nc.declare_dram_parameter("maskN", [QSH, K], I8, isOutput=False)
    ctx_o = nc.declare_dram_parameter("ctx", [QSH, D], BF16, isOutput=True)
    wts_o = nc.declare_dram_parameter("wts", [QSH, K], BF16, isOutput=True)
    e_dram = nc.dram_tensor("e_scratch", [1, K], BF16)

    AF = mybir.ActivationFunctionType
    ALU = mybir.AluOpType

    with TileContext(nc) as tc:
        with (
            tc.tile_pool(name="vpool", bufs=1) as vpool,
            tc.tile_pool(name="sval", bufs=1) as spool,
            tc.tile_pool(name="stats", bufs=1) as stats,
            tc.tile_pool(name="scr", bufs=2) as scr,
            tc.tile_pool(name="masks", bufs=1) as masks,
            tc.tile_pool(name="mepool", bufs=3) as mep,
            tc.tile_pool(name="wsb", bufs=3) as wsb,
            tc.tile_pool(name="csb", bufs=3) as csb,
            tc.tile_pool(name="small", bufs=8) as small,
            tc.tile_pool(name="psum", bufs=3, space="PSUM") as pp,
            tc.tile_pool(name="psumd", bufs=2, space="PSUM") as ppd,
        ):
            # Preload the ACT exp table while the first DMAs are in flight.
            warm = stats.tile([P, 1], F32, tag="warm")
            nc.vector.memset(warm[:], 0.0)
            nc.scalar.activation(warm[:], warm[:], AF.Exp)

            ks = stats.tile([P, KT], F32, tag="ks")
            e_f = stats.tile([P, KT], F32, tag="ef")
            e_b = stats.tile([P, KT], BF16, tag="eb")
            ebc = stats.tile([P, K], BF16, tag="ebc")

            mTa = masks.tile([P, KT, QSH], BF16, tag="mT")
            mT3 = maskT.rearrange("(t p) q -> p t q", p=P)
            mNa = masks.tile([P, QT, K], BF16, tag="mN")
            mN3 = maskN.rearrange("(j p) k -> p j k", p=P)

            # Stage A: per-tile {maskT, value} loads; ksum split ACT/DVE;
            # e = exp(ksum/32) in bf16; sval = e * value (bf16).
            v3 = value.rearrange("(t p) d -> t p d", p=P)
            vts, svs = [], []
            for t in range(KT):
                vt = vpool.tile([P, D], BF16, tag=f"v{t}")
                nc.sync.dma_start(out=vt[:], in_=v3[t])
                vts.append(vt)
                nc.gpsimd.dma_start(out=mTa[:, t, :], in_=mT3[:, t, :])
            for t in range(KT):
                if t % 2 == 0 and t > 0:
                    dummy = scr.tile([P, D], BF16, tag="scratch")
                    nc.scalar.activation(
                        dummy[:], vts[t][:], AF.Copy, accum_out=ks[:, t : t + 1]
                    )
                else:
                    nc.vector.tensor_reduce(
                        ks[:, t : t + 1], vts[t][:], axis=mybir.AxisListType.X,
                        op=ALU.add,
                    )
                nc.scalar.activation(
                    e_b[:, t : t + 1], ks[:, t : t + 1], AF.Exp, scale=1.0 / 32.0
                )
                nc.scalar.activation(
                    e_f[:, t : t + 1], ks[:, t : t + 1], AF.Exp, scale=1.0 / 32.0
                )
                sv = spool.tile([P, D], BF16, tag=f"s{t}")
                nc.vector.tensor_scalar_mul(sv[:], vts[t][:], e_f[:, t : t + 1])
                svs.append(sv)

            # maskN after the critical value+maskT stream (weights path only)
            for j in range(QT):
                nc.gpsimd.dma_start(out=mNa[:, j, :], in_=mN3[:, j, :])

            # ebc[q_part, kappa] = e broadcast along partitions, via a dense
            # DRAM bounce in kappa order (kappa = p*KT + t; host permutes the
            # weights path to match). gpsimd ring: off both input and output
            # HWDGE rings, so its e_b wait blocks no other traffic.
            nc.gpsimd.dma_start(
                out=e_dram[0, :].rearrange("(p t) -> p t", p=P), in_=e_b[:]
            )
            nc.gpsimd.dma_start(
                out=ebc[:], in_=e_dram[0:1, :].partition_broadcast(P)
            )

            # Stage B: context matmuls + PE-side denominators (N=1 matmuls,
            # ready exactly when each context PSUM completes, independent of
            # the late e-broadcast), 3-way interleaved ramp, weights path
            # (DVE multiply + ACT scale) threaded behind the evacuations.
            cts, dns, rs = {}, {}, {}

            def alloc_ct(j):
                cts[j] = (
                    pp.tile([P, 512], F32, tag="c0", name=f"c0_{j}"),
                    pp.tile([P, 512], F32, tag="c1", name=f"c1_{j}"),
                )

            def alloc_dn(j):
                dns[j] = ppd.tile([P, 1], F32, tag="dn", name=f"dn_{j}")

            def mm_ct(j, t):
                qs = slice(j * P, (j + 1) * P)
                st, sp = (t == 0), (t == KT - 1)
                c0, c1 = cts[j]
                lhsT = mTa[:, t, qs]
                nc.tensor.matmul(c0[:], lhsT, svs[t][:, 0:512], start=st, stop=sp)
                nc.tensor.matmul(c1[:], lhsT, svs[t][:, 512:1024], start=st, stop=sp)

            def mm_dn(j, t):
                qs = slice(j * P, (j + 1) * P)
                st, sp = (t == 0), (t == KT - 1)
                nc.tensor.matmul(
                    dns[j][:], mTa[:, t, qs], e_b[:, t : t + 1], start=st, stop=sp
                )

            def evac(j):
                qs = slice(j * P, (j + 1) * P)
                c0, c1 = cts.pop(j)
                dn = dns.pop(j)
                r = small.tile([P, 1], F32, tag="r", name=f"r_{j}")
                nc.vector.reciprocal(r[:], dn[:])
                cs = csb.tile([P, D], BF16, tag="cs", name=f"cs_{j}")
                nc.vector.tensor_scalar_mul(cs[:, 0:512], c0[:], r[:])
                nc.vector.tensor_scalar_mul(cs[:, 512:1024], c1[:], r[:])
                nc.scalar.dma_start(out=ctx_o[qs, :], in_=cs[:])
                rs[j] = r

            def wts_path(j, me=None):
                qs = slice(j * P, (j + 1) * P)
                if me is None:
                    me = mep.tile([P, K], BF16, tag="me", name=f"me_{j}")
                    nc.vector.tensor_tensor(
                        out=me[:], in0=mNa[:, j, :], in1=ebc[:], op=ALU.mult
                    )
                w = wsb.tile([P, K], BF16, tag="w", name=f"w_{j}")
                if j == QT - 1:
                    nc.vector.tensor_scalar_mul(w[:], me[:], rs[j][:])
                else:
                    nc.scalar.activation(w[:], me[:], AF.Copy, scale=rs[j][:])
                nc.scalar.dma_start(out=wts_o[qs, :], in_=w[:])

            RAMP = (0, 1, 2)
            for j in RAMP:
                alloc_ct(j)
            alloc_dn(0)
            alloc_dn(1)
            for t in range(KT):
                for j in RAMP:
                    mm_ct(j, t)
                    if j < 2:
                        mm_dn(j, t)
            alloc_dn(2)
            for t in range(KT):
                mm_dn(2, t)
            for j in RAMP:
                evac(j)
            for j in RAMP:
                wts_path(j)
            for j in range(3, QT):
                alloc_ct(j)
                alloc_dn(j)
                for t in range(KT):
                    mm_ct(j, t)
                    mm_dn(j, t)
                me = mep.tile([P, K], BF16, tag="me", name=f"me_{j}")
                nc.vector.tensor_tensor(
                    out=me[:], in0=mNa[:, j, :], in1=ebc[:], op=ALU.mult
                )
                evac(j)
                wts_path(j, me=me)

    nc.compile()
    return nc


def kernel(query, value, attention_mask):
    nc = _cache.get("nc")
    if nc is None:
        nc = _cache["nc"] = _build()

    value = np.asarray(value, dtype=np.float32)
    mask = np.asarray(attention_mask)

    # kappa-order: kappa = p*KT + t  <->  k = t*P + p. The weights path runs
    # in kappa order on-chip (dense e-vector bounce); host permutes maskN in
    # and un-permutes the wts output.
    kap = np.arange(K)
    k_of_kappa = (kap % KT) * P + kap // KT
    inv_kappa = (kap % P) * KT + kap // P  # kappa index of natural column k

    in_maps = []
    for c in range(NCORES):
        b, h = divmod(c, 2)
        msub = mask[b, h * QSH : (h + 1) * QSH, :]
        in_maps.append(
            {
                "value": value[b].astype(ml_dtypes.bfloat16),
                "maskT": np.ascontiguousarray(msub.T).astype(np.int8),
                "maskN": np.ascontiguousarray(
                    msub[:, k_of_kappa]
                ).astype(np.int8),
            }
        )

    res = run_bass_kernel_spmd(nc, in_maps, core_ids=list(range(NCORES)))
    _cache["last_results"] = res

    ctx = np.empty((B, Q, D), np.float32)
    wts = np.empty((B, Q, K), np.float32)
    for c in range(NCORES):
        b, h = divmod(c, 2)
        ctx[b, h * QSH : (h + 1) * QSH] = res.results[c]["ctx"].astype(np.float32)
        wts[b, h * QSH : (h + 1) * QSH] = (
            res.results[c]["wts"].astype(np.float32)[:, inv_kappa]
        )
    return ctx, wts



# revision 38
# speedup vs baseline: 1.0697x; 1.0403x over previous
"""Trainium2 Bass kernel for nn_AdditiveAttention (B=4, Q=K=2048, D=1024).

Math: scores[b,q,k] = (sum_d q[b,q,d] + sum_d v[b,k,d]) / sqrt(D) + mask bias.
The q-row term is constant along k, so it cancels in the softmax exactly:
    weights[b,q,k] = mask[b,q,k] * e[b,k] / denom[b,q]
    e[b,k]     = exp(sum_d value[b,k,d] / sqrt(D))
    denom[b,q] = sum_k mask[b,q,k] * e[b,k]
    context    = (mask @ (e * value)) / denom        (per batch)
(`query` only affects the reference output through float rounding.)

Sharding: 8 cores = (batch b in 0..3) x (query half h in 0..1); each core
computes a (1024, :) slice of both outputs for its batch. No collectives.

Compute dtype: bf16 matmul (the 0/1 mask operand is exactly representable),
fp32 accumulate in PSUM. Masks ship as int8 and are cast to bf16 in flight
by gpsimd (SWDGE) DMAs on their own ring; value ships bf16 on the sync ring;
outputs ship bf16 and are upcast to f32 on the host (~0.5% rel err vs the
2e-2 tolerance).

Orchestration notes (all engine streams are in-order; the layout below keeps
every cross-engine dependency off the critical path):
  - denominators come from N=1 PE matmuls accumulated alongside the context
    tiles (8 ns each), so PSUM evacuation never waits on the e-broadcast;
  - the e-vector broadcast bounces through DRAM in "kappa" order
    (kappa = p*KT + t) so both DMAs are dense; the host permutes the
    weights-path mask input and inverse-permutes the weights output;
  - the first three query tiles' accumulation chains are interleaved so the
    PE tracks the input DMA stream densely during the ramp (8 PSUM banks:
    3x c0 + 3x c1 + 2x dn).
Measured: ~87 us NEFF exec (neuron-profile), rel err ~3e-3.
"""

import os
import sys

import numpy as np
import ml_dtypes

for _p in ("/opt/trn_rl_repo", "/root/.axon_site/_ro/trn_rl_repo"):
    if os.path.isdir(_p) and _p not in sys.path:
        sys.path.append(_p)

import concourse.bacc as bacc
import concourse.mybir as mybir
from concourse.tile import TileContext
from concourse.bass_utils import run_bass_kernel_spmd

B, Q, K, D = 4, 2048, 2048, 1024
P = 128
NCORES = 8
QSH = Q // 2          # query rows per core
KT = K // P           # 16 k tiles
QT = QSH // P         # 8 q tiles per core
F32 = mybir.dt.float32
BF16 = mybir.dt.bfloat16
I8 = mybir.dt.int8

_cache = {}


def _build():
    nc = bacc.Bacc()
    value = nc.declare_dram_parameter("value", [K, D], BF16, isOutput=False)
    maskT = nc.declare_dram_parameter("maskT", [K, QSH], I8, isOutput=False)
    maskN = nc.declare_dram_parameter("maskN", [QSH, K], I8, isOutput=False)
    ctx_o = nc.declare_dram_parameter("ctx", [QSH, D], BF16, isOutput=True)
    wts_o = nc.declare_dram_parameter("wts", [QSH, K], BF16, isOutput=True)
    e_dram = nc.dram_tensor("e_scratch", [1, K], BF16)

    AF = mybir.ActivationFunctionType
    ALU = mybir.AluOpType

    with TileContext(nc) as tc:
        with (
            tc.tile_pool(name="vpool", bufs=1) as vpool,
            tc.tile_pool(name="sval", bufs=1) as spool,
            tc.tile_pool(name="stats", bufs=1) as stats,
            tc.tile_pool(name="scr", bufs=2) as scr,
            tc.tile_pool(name="masks", bufs=1) as masks,
            tc.tile_pool(name="mepool", bufs=3) as mep,
            tc.tile_pool(name="wsb", bufs=3) as wsb,
            tc.tile_pool(name="csb", bufs=3) as csb,
            tc.tile_pool(name="small", bufs=8) as small,
            tc.tile_pool(name="psum", bufs=3, space="PSUM") as pp,
            tc.tile_pool(name="psumd", bufs=2, space="PSUM") as ppd,
        ):
            # Preload the ACT exp table while the first DMAs are in flight.
            warm = stats.tile([P, 1], F32, tag="warm")
            nc.vector.memset(warm[:], 0.0)
            nc.scalar.activation(warm[:], warm[:], AF.Exp)

            ks = stats.tile([P, KT], F32, tag="ks")
            e_f = stats.tile([P, KT], F32, tag="ef")
            e_b = stats.tile([P, KT], BF16, tag="eb")
            ebc = stats.tile([P, K], BF16, tag="ebc")

            mTa = masks.tile([P, KT, QSH], BF16, tag="mT")
            mT3 = maskT.rearrange("(t p) q -> p t q", p=P)
            mNa = masks.tile([P, QT, K], BF16, tag="mN")
            mN3 = maskN.rearrange("(j p) k -> p j k", p=P)

            # Stage A: per-tile {maskT, value} loads; ksum split ACT/DVE;
            # e = exp(ksum/32) in bf16; sval = e * value (bf16).
            v3 = value.rearrange("(t p) d -> t p d", p=P)
            vts, svs = [], []
            for t in range(KT):
                vt = vpool.tile([P, D], BF16, tag=f"v{t}")
                nc.sync.dma_start(out=vt[:], in_=v3[t])
                vts.append(vt)
                nc.gpsimd.dma_start(out=mTa[:, t, :], in_=mT3[:, t, :])
            for t in range(KT):
                if t % 2 == 0 and t > 0:
                    dummy = scr.tile([P, D], BF16, tag="scratch")
                    nc.scalar.activation(
                        dummy[:], vts[t][:], AF.Copy, accum_out=ks[:, t : t + 1]
                    )
                else:
                    nc.vector.tensor_reduce(
                        ks[:, t : t + 1], vts[t][:], axis=mybir.AxisListType.X,
                        op=ALU.add,
                    )
                nc.scalar.activation(
                    e_b[:, t : t + 1], ks[:, t : t + 1], AF.Exp, scale=1.0 / 32.0
                )
                nc.scalar.activation(
                    e_f[:, t : t + 1], ks[:, t : t + 1], AF.Exp, scale=1.0 / 32.0
                )
                sv = spool.tile([P, D], BF16, tag=f"s{t}")
                nc.vector.tensor_scalar_mul(sv[:], vts[t][:], e_f[:, t : t + 1])
                svs.append(sv)

            # maskN after the critical value+maskT stream (weights path only)
            for j in range(QT):
                nc.gpsimd.dma_start(out=mNa[:, j, :], in_=mN3[:, j, :])

            # ebc[q_part, kappa] = e broadcast along partitions, via a dense
            # DRAM bounce in kappa order (kappa = p*KT + t; host permutes the
            # weights path to match). gpsimd ring: off both input and output
            # HWDGE rings, so its e_b wait blocks no other traffic.
            nc.gpsimd.dma_start(
                out=e_dram[0, :].rearrange("(p t) -> p t", p=P), in_=e_b[:]
            )
            nc.gpsimd.dma_start(
                out=ebc[:], in_=e_dram[0:1, :].partition_broadcast(P)
            )

            # Stage B: context matmuls + PE-side denominators (N=1 matmuls,
            # ready exactly when each context PSUM completes, independent of
            # the late e-broadcast), 3-way interleaved ramp, weights path
            # (DVE multiply + ACT scale) threaded behind the evacuations.
            cts, dns, rs = {}, {}, {}

            def alloc_ct(j):
                cts[j] = (
                    pp.tile([P, 512], F32, tag="c0", name=f"c0_{j}"),
                    pp.tile([P, 512], F32, tag="c1", name=f"c1_{j}"),
                )

            def alloc_dn(j):
                dns[j] = ppd.tile([P, 1], F32, tag="dn", name=f"dn_{j}")

            def mm_ct(j, t):
                qs = slice(j * P, (j + 1) * P)
                st, sp = (t == 0), (t == KT - 1)
                c0, c1 = cts[j]
                lhsT = mTa[:, t, qs]
                nc.tensor.matmul(c0[:], lhsT, svs[t][:, 0:512], start=st, stop=sp)
                nc.tensor.matmul(c1[:], lhsT, svs[t][:, 512:1024], start=st, stop=sp)

            def mm_dn(j, t):
                qs = slice(j * P, (j + 1) * P)
                st, sp = (t == 0), (t == KT - 1)
                nc.tensor.matmul(
                    dns[j][:], mTa[:, t, qs], e_b[:, t : t + 1], start=st, stop=sp
                )

            def evac(j):
                qs = slice(j * P, (j + 1) * P)
                c0, c1 = cts.pop(j)
                dn = dns.pop(j)
                r = small.tile([P, 1], F32, tag="r", name=f"r_{j}")
                nc.vector.reciprocal(r[:], dn[:])
                cs = csb.tile([P, D], BF16, tag="cs", name=f"cs_{j}")
                nc.vector.tensor_scalar_mul(cs[:, 0:512], c0[:], r[:])
                nc.vector.tensor_scalar_mul(cs[:, 512:1024], c1[:], r[:])
                nc.scalar.dma_start(out=ctx_o[qs, :], in_=cs[:])
                rs[j] = r

            def wts_path(j, me=None):
                qs = slice(j * P, (j + 1) * P)
                if me is None:
                    me = mep.tile([P, K], BF16, tag="me", name=f"me_{j}")
                    nc.vector.tensor_tensor(
                        out=me[:], in0=mNa[:, j, :], in1=ebc[:], op=ALU.mult
                    )
                w = wsb.tile([P, K], BF16, tag="w", name=f"w_{j}")
                if j == QT - 1:
                    nc.vector.tensor_scalar_mul(w[:], me[:], rs[j][:])
                else:
                    nc.scalar.activation(w[:], me[:], AF.Copy, scale=rs[j][:])
                nc.scalar.dma_start(out=wts_o[qs, :], in_=w[:])

            RAMP = (0, 1, 2)
            for j in RAMP:
                alloc_ct(j)
            alloc_dn(0)
            alloc_dn(1)
            for t in range(KT):
                for j in RAMP:
                    mm_ct(j, t)
                    if j < 2:
                        mm_dn(j, t)
            alloc_dn(2)
            for t in range(KT):
                mm_dn(2, t)
            for j in RAMP:
                evac(j)
            for j in RAMP:
                wts_path(j)
            for j in range(3, QT - 1):
                alloc_ct(j)
                alloc_dn(j)
                for t in range(KT):
                    mm_ct(j, t)
                    mm_dn(j, t)
                me = mep.tile([P, K], BF16, tag="me", name=f"me_{j}")
                nc.vector.tensor_tensor(
                    out=me[:], in0=mNa[:, j, :], in1=ebc[:], op=ALU.mult
                )
                evac(j)
                wts_path(j, me=me)

            # Last query tile: denominator from the fused DVE multiply+row-sum
            # instead of the PE chain, so the weights output completes
            # mid-kernel and only the context evacuation trails the last MM.
            jL = QT - 1
            alloc_ct(jL)
            me_l = mep.tile([P, K], BF16, tag="me", name=f"me_{jL}")
            den_l = small.tile([P, 1], F32, tag="den", name=f"den_{jL}")
            nc.vector.scalar_tensor_tensor(
                out=me_l[:], in0=mNa[:, jL, :], scalar=1.0, in1=ebc[:],
                op0=ALU.mult, op1=ALU.mult, accum_out=den_l[:],
            )
            r_l = small.tile([P, 1], F32, tag="r", name=f"r_{jL}")
            nc.vector.reciprocal(r_l[:], den_l[:])
            w_l = wsb.tile([P, K], BF16, tag="w", name=f"w_{jL}")
            nc.vector.tensor_scalar_mul(w_l[:], me_l[:], r_l[:])
            nc.scalar.dma_start(out=wts_o[jL * P : (jL + 1) * P, :], in_=w_l[:])
            for t in range(KT):
                mm_ct(jL, t)
            c0, c1 = cts.pop(jL)
            cs_l = csb.tile([P, D], BF16, tag="cs", name=f"cs_{jL}")
            nc.vector.tensor_scalar_mul(cs_l[:, 0:512], c0[:], r_l[:])
            nc.vector.tensor_scalar_mul(cs_l[:, 512:1024], c1[:], r_l[:])
            nc.scalar.dma_start(out=ctx_o[jL * P : (jL + 1) * P, :], in_=cs_l[:])

    nc.compile()
    return nc


def kernel(query, value, attention_mask):
    nc = _cache.get("nc")
    if nc is None:
        nc = _cache["nc"] = _build()

    value = np.asarray(value, dtype=np.float32)
    mask = np.asarray(attention_mask)

    # kappa-order: kappa = p*KT + t  <->  k = t*P + p. The weights path runs
    # in kappa order on-chip (dense e-vector bounce); host permutes maskN in
    # and un-permutes the wts output.
    kap = np.arange(K)
    k_of_kappa = (kap % KT) * P + kap // KT
    inv_kappa = (kap % P) * KT + kap // P  # kappa index of natural column k

    in_maps = []
    for c in range(NCORES):
        b, h = divmod(c, 2)
        msub = mask[b, h * QSH : (h + 1) * QSH, :]
        in_maps.append(
            {
                "value": value[b].astype(ml_dtypes.bfloat16),
                "maskT": np.ascontiguousarray(msub.T).astype(np.int8),
                "maskN": np.ascontiguousarray(
                    msub[:, k_of_kappa]
                ).astype(np.int8),
            }
        )

    res = run_bass_kernel_spmd(nc, in_maps, core_ids=list(range(NCORES)))
    _cache["last_results"] = res

    ctx = np.empty((B, Q, D), np.float32)
    wts = np.empty((B, Q, K), np.float32)
    for c in range(NCORES):
        b, h = divmod(c, 2)
        ctx[b, h * QSH : (h + 1) * QSH] = res.results[c]["ctx"].astype(np.float32)
        wts[b, h * QSH : (h + 1) * QSH] = (
            res.results[c]["wts"].astype(np.float32)[:, inv_kappa]
        )
    return ctx, wts



# revision 40
# speedup vs baseline: 1.0700x; 1.0003x over previous
"""Trainium2 Bass kernel for nn_AdditiveAttention (B=4, Q=K=2048, D=1024).

Math: scores[b,q,k] = (sum_d q[b,q,d] + sum_d v[b,k,d]) / sqrt(D) + mask bias.
The q-row term is constant along k, so it cancels in the softmax exactly:
    weights[b,q,k] = mask[b,q,k] * e[b,k] / denom[b,q]
    e[b,k]     = exp(sum_d value[b,k,d] / sqrt(D))
    denom[b,q] = sum_k mask[b,q,k] * e[b,k]
    context    = (mask @ (e * value)) / denom        (per batch)
(`query` only affects the reference output through float rounding.)

Sharding: 8 cores = (batch b in 0..3) x (query half h in 0..1); each core
computes a (1024, :) slice of both outputs for its batch. No collectives.

Compute dtype: bf16 matmul (the 0/1 mask operand is exactly representable),
fp32 accumulate in PSUM. Masks ship as int8 and are cast to bf16 in flight
by gpsimd (SWDGE) DMAs on their own ring; value ships bf16 on the sync ring;
outputs ship bf16 and are upcast to f32 on the host (~0.5% rel err vs the
2e-2 tolerance).

Orchestration notes (all engine streams are in-order; the layout below keeps
every cross-engine dependency off the critical path):
  - denominators come from N=1 PE matmuls accumulated alongside the context
    tiles (8 ns each), so PSUM evacuation never waits on the e-broadcast;
  - the e-vector broadcast bounces through DRAM in "kappa" order
    (kappa = p*KT + t) so both DMAs are dense; the host permutes the
    weights-path mask input and inverse-permutes the weights output;
  - the first three query tiles' accumulation chains are interleaved so the
    PE tracks the input DMA stream densely during the ramp (8 PSUM banks:
    3x c0 + 3x c1 + 2x dn);
  - the last query tile's denominator comes from the fused DVE
    multiply+row-sum instead, so its weights output completes before the
    final matmuls and only the context evacuation trails the last MM.
Measured: ~83.5 us NEFF exec (neuron-profile), rel err ~3e-3.
"""

import os
import sys

import numpy as np
import ml_dtypes

for _p in ("/opt/trn_rl_repo", "/root/.axon_site/_ro/trn_rl_repo"):
    if os.path.isdir(_p) and _p not in sys.path:
        sys.path.append(_p)

import concourse.bacc as bacc
import concourse.mybir as mybir
from concourse.tile import TileContext
from concourse.bass_utils import run_bass_kernel_spmd

B, Q, K, D = 4, 2048, 2048, 1024
P = 128
NCORES = 8
QSH = Q // 2          # query rows per core
KT = K // P           # 16 k tiles
QT = QSH // P         # 8 q tiles per core
F32 = mybir.dt.float32
BF16 = mybir.dt.bfloat16
I8 = mybir.dt.int8

_cache = {}


def _build():
    nc = bacc.Bacc()
    value = nc.declare_dram_parameter("value", [K, D], BF16, isOutput=False)
    maskT = nc.declare_dram_parameter("maskT", [K, QSH], I8, isOutput=False)
    maskN = nc.declare_dram_parameter("maskN", [QSH, K], I8, isOutput=False)
    ctx_o = nc.declare_dram_parameter("ctx", [QSH, D], BF16, isOutput=True)
    wts_o = nc.declare_dram_parameter("wts", [QSH, K], BF16, isOutput=True)
    e_dram = nc.dram_tensor("e_scratch", [1, K], BF16)

    AF = mybir.ActivationFunctionType
    ALU = mybir.AluOpType

    with TileContext(nc) as tc:
        with (
            tc.tile_pool(name="vpool", bufs=1) as vpool,
            tc.tile_pool(name="sval", bufs=1) as spool,
            tc.tile_pool(name="stats", bufs=1) as stats,
            tc.tile_pool(name="scr", bufs=2) as scr,
            tc.tile_pool(name="masks", bufs=1) as masks,
            tc.tile_pool(name="mepool", bufs=3) as mep,
            tc.tile_pool(name="wsb", bufs=3) as wsb,
            tc.tile_pool(name="csb", bufs=3) as csb,
            tc.tile_pool(name="small", bufs=8) as small,
            tc.tile_pool(name="psum", bufs=3, space="PSUM") as pp,
            tc.tile_pool(name="psumd", bufs=2, space="PSUM") as ppd,
        ):
            ks = stats.tile([P, KT], F32, tag="ks")
            e_f = stats.tile([P, KT], F32, tag="ef")
            e_b = stats.tile([P, KT], BF16, tag="eb")
            ebc = stats.tile([P, K], BF16, tag="ebc")

            mTa = masks.tile([P, KT, QSH], BF16, tag="mT")
            mT3 = maskT.rearrange("(t p) q -> p t q", p=P)
            mNa = masks.tile([P, QT, K], BF16, tag="mN")
            mN3 = maskN.rearrange("(j p) k -> p j k", p=P)

            # Stage A: per-tile {maskT, value} loads; ksum split ACT/DVE;
            # e = exp(ksum/32) in bf16; sval = e * value (bf16).
            v3 = value.rearrange("(t p) d -> t p d", p=P)
            vts, svs = [], []
            for t in range(KT):
                vt = vpool.tile([P, D], BF16, tag=f"v{t}")
                nc.sync.dma_start(out=vt[:], in_=v3[t])
                vts.append(vt)
                nc.gpsimd.dma_start(out=mTa[:, t, :], in_=mT3[:, t, :])
            # Preload the ACT exp table while the input DMAs stream (emitted
            # after the DMA issues so it cannot delay the first input bytes).
            warm = stats.tile([P, 1], F32, tag="warm")
            nc.vector.memset(warm[:], 0.0)
            nc.scalar.activation(warm[:], warm[:], AF.Exp)

            for t in range(KT):
                if t % 2 == 0 and t > 0:
                    dummy = scr.tile([P, D], BF16, tag="scratch")
                    nc.scalar.activation(
                        dummy[:], vts[t][:], AF.Copy, accum_out=ks[:, t : t + 1]
                    )
                else:
                    nc.vector.tensor_reduce(
                        ks[:, t : t + 1], vts[t][:], axis=mybir.AxisListType.X,
                        op=ALU.add,
                    )
                nc.scalar.activation(
                    e_b[:, t : t + 1], ks[:, t : t + 1], AF.Exp, scale=1.0 / 32.0
                )
                nc.scalar.activation(
                    e_f[:, t : t + 1], ks[:, t : t + 1], AF.Exp, scale=1.0 / 32.0
                )
                sv = spool.tile([P, D], BF16, tag=f"s{t}")
                nc.vector.tensor_scalar_mul(sv[:], vts[t][:], e_f[:, t : t + 1])
                svs.append(sv)

            # maskN after the critical value+maskT stream (weights path only)
            for j in range(QT):
                nc.gpsimd.dma_start(out=mNa[:, j, :], in_=mN3[:, j, :])

            # ebc[q_part, kappa] = e broadcast along partitions, via a dense
            # DRAM bounce in kappa order (kappa = p*KT + t; host permutes the
            # weights path to match). gpsimd ring: off both input and output
            # HWDGE rings, so its e_b wait blocks no other traffic.
            nc.gpsimd.dma_start(
                out=e_dram[0, :].rearrange("(p t) -> p t", p=P), in_=e_b[:]
            )
            nc.gpsimd.dma_start(
                out=ebc[:], in_=e_dram[0:1, :].partition_broadcast(P)
            )

            # Stage B: context matmuls + PE-side denominators (N=1 matmuls,
            # ready exactly when each context PSUM completes, independent of
            # the late e-broadcast), 3-way interleaved ramp, weights path
            # (DVE multiply + ACT scale) threaded behind the evacuations.
            cts, dns, rs = {}, {}, {}

            def alloc_ct(j):
                cts[j] = (
                    pp.tile([P, 512], F32, tag="c0", name=f"c0_{j}"),
                    pp.tile([P, 512], F32, tag="c1", name=f"c1_{j}"),
                )

            def alloc_dn(j):
                dns[j] = ppd.tile([P, 1], F32, tag="dn", name=f"dn_{j}")

            def mm_ct(j, t):
                qs = slice(j * P, (j + 1) * P)
                st, sp = (t == 0), (t == KT - 1)
                c0, c1 = cts[j]
                lhsT = mTa[:, t, qs]
                nc.tensor.matmul(c0[:], lhsT, svs[t][:, 0:512], start=st, stop=sp)
                nc.tensor.matmul(c1[:], lhsT, svs[t][:, 512:1024], start=st, stop=sp)

            def mm_dn(j, t):
                qs = slice(j * P, (j + 1) * P)
                st, sp = (t == 0), (t == KT - 1)
                nc.tensor.matmul(
                    dns[j][:], mTa[:, t, qs], e_b[:, t : t + 1], start=st, stop=sp
                )

            def evac(j):
                qs = slice(j * P, (j + 1) * P)
                c0, c1 = cts.pop(j)
                dn = dns.pop(j)
                r = small.tile([P, 1], F32, tag="r", name=f"r_{j}")
                nc.vector.reciprocal(r[:], dn[:])
                cs = csb.tile([P, D], BF16, tag="cs", name=f"cs_{j}")
                nc.vector.tensor_scalar_mul(cs[:, 0:512], c0[:], r[:])
                nc.vector.tensor_scalar_mul(cs[:, 512:1024], c1[:], r[:])
                nc.scalar.dma_start(out=ctx_o[qs, :], in_=cs[:])
                rs[j] = r

            def wts_path(j, me=None):
                qs = slice(j * P, (j + 1) * P)
                if me is None:
                    me = mep.tile([P, K], BF16, tag="me", name=f"me_{j}")
                    nc.vector.tensor_tensor(
                        out=me[:], in0=mNa[:, j, :], in1=ebc[:], op=ALU.mult
                    )
                w = wsb.tile([P, K], BF16, tag="w", name=f"w_{j}")
                if j == QT - 1:
                    nc.vector.tensor_scalar_mul(w[:], me[:], rs[j][:])
                else:
                    nc.scalar.activation(w[:], me[:], AF.Copy, scale=rs[j][:])
                nc.scalar.dma_start(out=wts_o[qs, :], in_=w[:])

            RAMP = (0, 1, 2)
            for j in RAMP:
                alloc_ct(j)
            alloc_dn(0)
            alloc_dn(1)
            for t in range(KT):
                for j in RAMP:
                    mm_ct(j, t)
                    if j < 2:
                        mm_dn(j, t)
            alloc_dn(2)
            for t in range(KT):
                mm_dn(2, t)
            for j in RAMP:
                evac(j)
            for j in RAMP:
                wts_path(j)
            for j in range(3, QT - 1):
                alloc_ct(j)
                alloc_dn(j)
                for t in range(KT):
                    mm_ct(j, t)
                    mm_dn(j, t)
                me = mep.tile([P, K], BF16, tag="me", name=f"me_{j}")
                nc.vector.tensor_tensor(
                    out=me[:], in0=mNa[:, j, :], in1=ebc[:], op=ALU.mult
                )
                evac(j)
                wts_path(j, me=me)

            # Last query tile: denominator from the fused DVE multiply+row-sum
            # instead of the PE chain, so the weights output completes
            # mid-kernel and only the context evacuation trails the last MM.
            jL = QT - 1
            alloc_ct(jL)
            me_l = mep.tile([P, K], BF16, tag="me", name=f"me_{jL}")
            den_l = small.tile([P, 1], F32, tag="den", name=f"den_{jL}")
            nc.vector.scalar_tensor_tensor(
                out=me_l[:], in0=mNa[:, jL, :], scalar=1.0, in1=ebc[:],
                op0=ALU.mult, op1=ALU.mult, accum_out=den_l[:],
            )
            r_l = small.tile([P, 1], F32, tag="r", name=f"r_{jL}")
            nc.vector.reciprocal(r_l[:], den_l[:])
            w_l = wsb.tile([P, K], BF16, tag="w", name=f"w_{jL}")
            nc.vector.tensor_scalar_mul(w_l[:], me_l[:], r_l[:])
            nc.scalar.dma_start(out=wts_o[jL * P : (jL + 1) * P, :], in_=w_l[:])
            for t in range(KT):
                mm_ct(jL, t)
            c0, c1 = cts.pop(jL)
            cs_l = csb.tile([P, D], BF16, tag="cs", name=f"cs_{jL}")
            nc.vector.tensor_scalar_mul(cs_l[:, 0:512], c0[:], r_l[:])
            nc.vector.tensor_scalar_mul(cs_l[:, 512:1024], c1[:], r_l[:])
            nc.scalar.dma_start(out=ctx_o[jL * P : (jL + 1) * P, :], in_=cs_l[:])

    nc.compile()
    return nc


def kernel(query, value, attention_mask):
    nc = _cache.get("nc")
    if nc is None:
        nc = _cache["nc"] = _build()

    value = np.asarray(value, dtype=np.float32)
    mask = np.asarray(attention_mask)

    # kappa-order: kappa = p*KT + t  <->  k = t*P + p. The weights path runs
    # in kappa order on-chip (dense e-vector bounce); host permutes maskN in
    # and un-permutes the wts output.
    kap = np.arange(K)
    k_of_kappa = (kap % KT) * P + kap // KT
    inv_kappa = (kap % P) * KT + kap // P  # kappa index of natural column k

    in_maps = []
    for c in range(NCORES):
        b, h = divmod(c, 2)
        msub = mask[b, h * QSH : (h + 1) * QSH, :]
        in_maps.append(
            {
                "value": value[b].astype(ml_dtypes.bfloat16),
                "maskT": np.ascontiguousarray(msub.T).astype(np.int8),
                "maskN": np.ascontiguousarray(
                    msub[:, k_of_kappa]
                ).astype(np.int8),
            }
        )

    res = run_bass_kernel_spmd(nc, in_maps, core_ids=list(range(NCORES)))
    _cache["last_results"] = res

    ctx = np.empty((B, Q, D), np.float32)
    wts = np.empty((B, Q, K), np.float32)
    for c in range(NCORES):
        b, h = divmod(c, 2)
        ctx[b, h * QSH : (h + 1) * QSH] = res.results[c]["ctx"].astype(np.float32)
        wts[b, h * QSH : (h + 1) * QSH] = (
            res.results[c]["wts"].astype(np.float32)[:, inv_kappa]
        )
    return ctx, wts

